# revision 27
# baseline (speedup 1.0000x reference)
"""AggAttn Trainium2 kernel: fused dual-score attention across 8 NeuronCores.

Sharding: core c handles batch b = c // 2 and heads (c % 2) * 8 .. + 8.
Host pre-transposes per-core inputs, folds the scalar gates into the
projection weights, launches one SPMD Bass program on cores 0-7, and
reassembles / reduces the partial outputs.

Math (per batch b, head h):
  qh = q @ Wq.T + bq   (per-head slices, [L, hd])
  S  = lam_cross/8 * qh @ kh.T + lam_self/8 * kh @ vh.T        [L, L]
  attn = softmax(S)  (returned as output 2)
  out  = concat_h(attn @ vh) @ Wo.T + bo                       [L, D]

Device-side formulation per head:
  A = [c1*qh^T ; kh^T]  (stacked [128, L]),  B = [kh^T ; c2*vh^T]
  S  chunk  = A[:, lc]^T @ B      (one K=128 matmul pair, both terms fused)
  S^T chunk = B[:, mc]^T @ A      (same product, transposed layout)
  exp(S) -> rowsum (ACT accum) -> attn = exp(S)/rowsum -> DRAM
  pv = vh_chunks^T-free PV: attn_out^T[hd, L] = sum_m (c2*vh)[m,:]^T exp(S^T)[m,:]
  attn_out^T normalized by 1/(c2*rowsum) broadcast, then row-parallel out-proj.
"""

import numpy as np

B, L, D, H, HD = 4, 1024, 1024, 16, 64
NCORES = 8
HPC = 8  # heads per core
PAIRS = 4  # head pairs per core
NT = 8  # 128-row tiles per 1024
F32NP = np.float32

# test.py knobs (the grading harness just calls kernel())
TRACE = False
LAST_RESULT = None  # BassKernelResults of the last run


def _ensure_ntff_hook():
    """Provide antenv.axon_hooks when the image lacks it, wiring the
    NTFF profile hook straight to libaxon_pjrt.so (slim copy of the
    trn_boot._ntff_profile_via_ctypes path)."""
    import contextlib
    import ctypes
    import sys
    import types

    try:
        from antenv.axon_hooks import get_axon_ntff_profile_hook  # noqa: F401
        return
    except ImportError:
        pass
    import antenv

    mod = types.ModuleType("antenv.axon_hooks")
    holder = [None]
    mod.set_axon_ntff_profile_hook = lambda h: holder.__setitem__(0, h)
    mod.get_axon_ntff_profile_hook = lambda: holder[0]
    sys.modules["antenv.axon_hooks"] = mod
    antenv.axon_hooks = mod

    so_path = "/opt/axon/libaxon_pjrt.so"
    try:
        lib = ctypes.CDLL(so_path)
    except OSError:
        return
    if not hasattr(lib, "axon_start_nrt_profile"):
        return
    lib.axon_start_nrt_profile.argtypes = [ctypes.POINTER(ctypes.c_int64),
                                           ctypes.c_size_t]
    lib.axon_start_nrt_profile.restype = ctypes.c_int64
    lib.axon_stop_nrt_profile.argtypes = [ctypes.c_char_p]
    lib.axon_stop_nrt_profile.restype = ctypes.c_int64

    @contextlib.contextmanager
    def _hook(output_dir, device_ids):
        import jax

        jax.devices()
        if device_ids:
            ids = (ctypes.c_int64 * len(device_ids))(*device_ids)
            rc = lib.axon_start_nrt_profile(ids, len(device_ids))
        else:
            rc = lib.axon_start_nrt_profile(None, 0)
        if rc != 0:
            raise RuntimeError(f"axon_start_nrt_profile rc={rc}")
        try:
            yield
        finally:
            n = lib.axon_stop_nrt_profile(str(output_dir).encode())
            print(f"ntff profile: {n} file(s) -> {output_dir}")

    mod.set_axon_ntff_profile_hook(_hook)


def _patch_tile_drain():
    """This image's walrus rejects instructions with >2 sync waits; Tile's
    kernel-tail drain collects one wait per live producer. Split them into
    a chain of single-wait nops on SyncE before the drain."""
    import concourse.tile as tile_mod
    from concourse.vector_clock import ScopedClock
    from concourse import mybir

    if getattr(tile_mod.TileContext, "_drain_split_patch", False):
        return

    def _drain_and_barrier(self, tick_clock, wait_clock):
        nc = self.nc
        w_nop = nc.sync.nop(nofuse=True, hint="drain_waits")
        wait_clock.add_sem_waits(
            w_nop.ins, ScopedClock({None: tick_clock.global_clock}))
        si = w_nop.ins.sync_info
        if si is not None and si.on_wait and len(si.on_wait) > 1:
            waits = list(si.on_wait)
            w_nop.ins.sync_info = mybir.SyncInfo(
                on_wait=waits[:1], on_update=list(si.on_update))
            for w in waits[1:]:
                n2 = nc.sync.nop(nofuse=True, hint="drain_waits")
                n2.ins.sync_info = mybir.SyncInfo(on_wait=[w], on_update=[])
        nc.sync.drain()
        nc.all_engine_barrier()
        assert self.sems is not None
        popped = nc._tile_sem_poison_stack.pop()
        assert popped is self._sem_poison
        nc.clear_and_free_semaphores(list(self.sems.allocated().values()))
        nc.all_engine_barrier()

    tile_mod.TileContext._drain_and_barrier = _drain_and_barrier
    tile_mod.TileContext._drain_split_patch = True

    # This walrus also caps EVERY instruction at one sync wait. Before
    # lowering, hoist excess waits onto same-engine single-wait nops
    # inserted immediately before the overloaded instruction.
    import bass_rust

    _orig_loi = tile_mod.TileContext._lower_ordered_insts
    _ctr = [0]

    def _split_waits_loi(self, ordered):
        for insts in ordered.values():
            out = []
            for inst in insts:
                si = getattr(inst, "sync_info", None)
                if (si is not None and si.on_wait and len(si.on_wait) > 1
                        and not hasattr(inst, "child_blocks")
                        and type(inst).__name__ not in
                        ("TileBranchInst", "BassTileLoopBlock")):
                    waits = list(si.on_wait)
                    for w in waits[:-1]:
                        _ctr[0] += 1
                        nop = bass_rust.InstNoOp(
                            name=f"I-wsplit{_ctr[0]}", ins=[], outs=[])
                        nop.engine = inst.engine
                        nop.bass_nofuse = True
                        nop.sync_info = mybir.SyncInfo(on_wait=[w],
                                                       on_update=[])
                        out.append(nop)
                    inst.sync_info = mybir.SyncInfo(
                        on_wait=[waits[-1]], on_update=list(si.on_update))
                out.append(inst)
            insts[:] = out
        return _orig_loi(self, ordered)

    tile_mod.TileContext._lower_ordered_insts = _split_waits_loi


def _build_program(inv_c2: float):
    import concourse.bass as bass
    import concourse.tile as tile
    from concourse import mybir
    from contextlib import ExitStack

    _patch_tile_drain()

    F32 = mybir.dt.float32
    F32R = mybir.dt.float32r
    AF = mybir.ActivationFunctionType
    OP = mybir.AluOpType

    nc = bass.Bass("TRN2", target_bir_lowering=False, debug=False,
                   num_devices=NCORES)

    xqT = nc.dram_tensor("xqT", [D, L], F32R, kind="ExternalInput").ap()
    xkT = nc.dram_tensor("xkT", [D, L], F32R, kind="ExternalInput").ap()
    xvT = nc.dram_tensor("xvT", [D, L], F32R, kind="ExternalInput").ap()
    wq = nc.dram_tensor("wq", [D, HPC * HD], F32R, kind="ExternalInput").ap()
    wk = nc.dram_tensor("wk", [D, HPC * HD], F32R, kind="ExternalInput").ap()
    wv = nc.dram_tensor("wv", [D, HPC * HD], F32R, kind="ExternalInput").ap()
    woT = nc.dram_tensor("woT", [HPC * HD, D], F32R, kind="ExternalInput").ap()
    bq_d = nc.dram_tensor("bq_d", [HPC * HD], F32, kind="ExternalInput").ap()
    bk_d = nc.dram_tensor("bk_d", [HPC * HD], F32, kind="ExternalInput").ap()
    bv_d = nc.dram_tensor("bv_d", [HPC * HD], F32, kind="ExternalInput").ap()
    ident_d = nc.dram_tensor("ident_d", [128, 128], F32,
                             kind="ExternalInput").ap()

    attn_w = nc.dram_tensor("attn_w", [HPC, L, L], F32,
                            kind="ExternalOutput").ap()
    outp = nc.dram_tensor("outp", [L, D], F32, kind="ExternalOutput").ap()

    with tile.TileContext(nc) as tc:
        with ExitStack() as ctx:
            xt = ctx.enter_context(tc.tile_pool(name="xt", bufs=15))
            wp = ctx.enter_context(tc.tile_pool(name="wp", bufs=8))
            stack = ctx.enter_context(tc.tile_pool(name="stack", bufs=1))
            vhp = ctx.enter_context(tc.tile_pool(name="vhp", bufs=1))
            wop = ctx.enter_context(tc.tile_pool(name="wop", bufs=1))
            aotp = ctx.enter_context(tc.tile_pool(name="aotp", bufs=1))
            small = ctx.enter_context(tc.tile_pool(name="small", bufs=2))
            replp = ctx.enter_context(tc.tile_pool(name="replp", bufs=2))
            const = ctx.enter_context(tc.tile_pool(name="const", bufs=1))
            ps = ctx.enter_context(
                tc.tile_pool(name="ps", bufs=3, space="PSUM"))
            ps_pv = ctx.enter_context(
                tc.tile_pool(name="ps_pv", bufs=1, space="PSUM"))

            # constants
            ident_t = const.tile([128, 128], F32, tag="ident", name="ident_t")
            nc.sync.dma_start(ident_t[:], ident_d[:])
            # PE warmup: ~17us of dummy matmuls during the initial input
            # loads, so HAM is at full clock when the first projection runs
            warm_ps = ps.tile([128, 128], F32, tag="ps", name="warm_ps")
            for _ in range(40):
                nc.tensor.matmul(warm_ps[:], ident_t[:], ident_t[:],
                                 start=True, stop=True)
            bq_t = const.tile([128, PAIRS], F32, tag="bq", name="bq_t")
            nc.sync.dma_start(bq_t[:], bq_d.rearrange("(a p) -> p a", p=128))
            bk_t = const.tile([128, PAIRS], F32, tag="bk", name="bk_t")
            nc.sync.dma_start(bk_t[:], bk_d.rearrange("(a p) -> p a", p=128))
            bv_t = const.tile([128, PAIRS], F32, tag="bv", name="bv_t")
            nc.sync.dma_start(bv_t[:], bv_d.rearrange("(a p) -> p a", p=128))

            # persistent tiles
            A = [stack.tile([128, L], F32R, tag=f"A{h}", name=f"A{h}")
                 for h in range(HPC)]
            Bs = [stack.tile([128, L], F32R, tag=f"B{h}", name=f"B{h}")
                  for h in range(HPC)]
            vh = [vhp.tile([128, HPC * HD], F32R, tag=f"vh{i}", name=f"vh{i}")
                  for i in range(NT)]
            aot = [aotp.tile([128, L], F32R, tag=f"aot{i}", name=f"aot{i}")
                   for i in range(PAIRS)]
            wo_t = [wop.tile([128, L], F32R, tag=f"wo{i}", name=f"wo{i}")
                    for i in range(PAIRS)]

            def load_xw(x_ap, w_ap, nm):
                # interleave x (SP queue) and w (ACT queue) chunk loads so
                # the first projection matmul can start after chunk 0 lands
                xs, ws = [], []
                for kc in range(NT):
                    t = xt.tile([128, L], F32R, tag="xt", name=f"x{nm}{kc}")
                    nc.sync.dma_start(t[:], x_ap[kc * 128:(kc + 1) * 128, :])
                    xs.append(t)
                    t2 = wp.tile([128, HPC * HD], F32R, tag="w",
                                 name=f"w{nm}{kc}")
                    nc.scalar.dma_start(t2[:],
                                        w_ap[kc * 128:(kc + 1) * 128, :])
                    ws.append(t2)
                return xs, ws

            def proj_pair(xts, wts, p, name):
                pp = ps.tile([128, L], F32, tag="ps", name=f"pp{name}{p}")
                for kc in range(NT):
                    st, sp_ = (kc == 0), (kc == NT - 1)
                    wsl = wts[kc][:, p * 128:(p + 1) * 128]
                    nc.tensor.matmul(pp[:, 0:512], wsl, xts[kc][:, 0:512],
                                     start=st, stop=sp_)
                    nc.tensor.matmul(pp[:, 512:1024], wsl,
                                     xts[kc][:, 512:1024],
                                     start=st, stop=sp_)
                return pp

            # ---- Phase A: projections into stacks ----
            xq_ts, wq_ts = load_xw(xqT, wq, "q")
            for p in range(PAIRS):
                pp = proj_pair(xq_ts, wq_ts, p, "q")
                h0, h1 = 2 * p, 2 * p + 1
                nc.vector.tensor_scalar_add(A[h0][0:64, :], pp[0:64, :],
                                            bq_t[0:64, p:p + 1])
                nc.vector.tensor_scalar_add(A[h1][64:128, :], pp[64:128, :],
                                            bq_t[64:128, p:p + 1])

            xk_ts, wk_ts = load_xw(xkT, wk, "k")
            for p in range(PAIRS):
                pp = proj_pair(xk_ts, wk_ts, p, "k")
                h0, h1 = 2 * p, 2 * p + 1
                nc.vector.tensor_scalar_add(Bs[h0][0:64, :], pp[0:64, :],
                                            bk_t[0:64, p:p + 1])
                nc.vector.tensor_scalar_add(Bs[h1][64:128, :], pp[64:128, :],
                                            bk_t[64:128, p:p + 1])
                # kh also needed in the A stacks (other partition half):
                nc.gpsimd.dma_start(A[h0][64:128, :], Bs[h0][0:64, :])
                nc.gpsimd.dma_start(A[h1][0:64, :], Bs[h1][64:128, :])

            # v: wv columns are pair-swapped on the host (head 2p+1 first)
            xv_ts, wv_ts = load_xw(xvT, wv, "v")
            for p in range(PAIRS):
                pp = proj_pair(xv_ts, wv_ts, p, "v1")
                h0, h1 = 2 * p, 2 * p + 1
                nc.vector.tensor_scalar_add(Bs[h1][0:64, :], pp[0:64, :],
                                            bv_t[0:64, p:p + 1])
                nc.vector.tensor_scalar_add(Bs[h0][64:128, :], pp[64:128, :],
                                            bv_t[64:128, p:p + 1])

            # v again, in [m, hd] layout for the PV matmul (no bias; the
            # bias term is exact on the host: softmax rows sum to 1).
            # Emitted inside head 0 (after its l-loop) so it overlaps the
            # l-loop's ACT work instead of delaying attention start.
            def emit_v2():
                for mc in range(NT):
                    pp = ps.tile([128, L], F32, tag="ps", name=f"ppv2{mc}")
                    for kc in range(NT):
                        nc.tensor.matmul(
                            pp[:, 0:512],
                            xv_ts[kc][:, mc * 128:(mc + 1) * 128],
                            wv_ts[kc][:],
                            start=(kc == 0), stop=(kc == NT - 1))
                    nc.vector.tensor_copy(vh[mc][:], pp[:, 0:512])

            emit_v2()

            for i in range(PAIRS):
                nc.scalar.dma_start(wo_t[i][:], woT[i * 128:(i + 1) * 128, :])

            # ---- Phase B: per-head attention ----
            # Interleaved chunk loop: S(lc) + S^T(mc) + PV(mc) together so
            # the PE stream stays dense (6 matmuls per chunk vs 2 exps on
            # ACT) and HAM stays at full clock.
            # finalize(h) = normalize attn_out^T and place into aot. Emitted
            # one head LATE (software pipeline) so the DVE's in-order stream
            # isn't blocked waiting for head h's repl DMA while head h+1's
            # l-loop DVE work is ready.
            pending_fin = [None]

            def emit_finalize():
                fin = pending_fin[0]
                if fin is None:
                    return
                pv_prev, repl_prev, hprev = fin
                pair_i, half = hprev // 2, hprev % 2
                if half == 0:
                    nc.vector.tensor_mul(aot[pair_i][0:64, :], pv_prev[:],
                                         repl_prev[:])
                else:
                    tmp = xt.tile([64, L], F32R, tag="xt",
                                  name=f"tmp{hprev}")
                    nc.vector.tensor_mul(tmp[:], pv_prev[:], repl_prev[:])
                    nc.gpsimd.dma_start(aot[pair_i][64:128, :], tmp[:])
                pending_fin[0] = None

            for h in range(HPC):
                A_, B_ = A[h], Bs[h]
                rs_t = small.tile([128, NT], F32, tag="rs", name=f"rs{h}")
                ri_t = small.tile([128, NT], F32, tag="ri", name=f"ri{h}")
                vcol = (h ^ 1) * HD
                with nc.named_scope(f"head{h}"):
                    # previous head's finalize first: its repl DMA finished
                    # long ago; running it now clears the pv slot and DVE
                    # stream before this head needs them
                    emit_finalize()
                    pv_ps = ps_pv.tile([64, L], F32, tag="pv", name=f"pv{h}")
                    # interleaved chunks: S(i) for the attn output plus
                    # S^T(i)+PV(i), so psum slot recycling never bunches at
                    # the head boundary
                    for i in range(NT):
                        sp_ = ps.tile([128, L], F32, tag="ps",
                                      name=f"s{h}_{i}")
                        asl = A_[:, i * 128:(i + 1) * 128]
                        nc.tensor.matmul(sp_[:, 0:512], asl, B_[:, 0:512],
                                         start=True, stop=True)
                        nc.tensor.matmul(sp_[:, 512:1024], asl,
                                         B_[:, 512:1024],
                                         start=True, stop=True)
                        ex = xt.tile([128, L], F32, tag="xt",
                                       name=f"ex{h}_{i}")
                        nc.scalar.activation(ex[:], sp_[:], AF.Exp,
                                             accum_out=rs_t[:, i:i + 1])
                        nc.vector.reciprocal(ri_t[:, i:i + 1],
                                             rs_t[:, i:i + 1])
                        at = xt.tile([128, L], F32, tag="xt",
                                       name=f"at{h}_{i}")
                        nc.vector.tensor_scalar_mul(at[:], ex[:],
                                                    ri_t[:, i:i + 1])
                        nc.sync.dma_start(
                            attn_w[h, i * 128:(i + 1) * 128, :], at[:])

                        stp = ps.tile([128, L], F32, tag="ps",
                                      name=f"st{h}_{i}")
                        bsl = B_[:, i * 128:(i + 1) * 128]
                        nc.tensor.matmul(stp[:, 0:512], bsl, A_[:, 0:512],
                                         start=True, stop=True)
                        nc.tensor.matmul(stp[:, 512:1024], bsl,
                                         A_[:, 512:1024],
                                         start=True, stop=True)
                        ext = xt.tile([128, L], F32R, tag="xt",
                                        name=f"ext{h}_{i}")
                        nc.scalar.activation(ext[:], stp[:], AF.Exp)
                        vsl = vh[i][:, vcol:vcol + HD]
                        nc.tensor.matmul(pv_ps[:, 0:512], vsl, ext[:, 0:512],
                                         start=(i == 0), stop=(i == NT - 1))
                        nc.tensor.matmul(pv_ps[:, 512:1024], vsl,
                                         ext[:, 512:1024],
                                         start=(i == 0), stop=(i == NT - 1))

                    # 1/(c2*rowsum) -> [64, L] replica; small DMAs ride the
                    # idle SWDGE ring so they never queue behind attn writes
                    ri2 = small.tile([128, NT], F32, tag="ri2",
                                     name=f"ri2_{h}")
                    nc.vector.tensor_scalar_mul(ri2[:], ri_t[:],
                                                float(inv_c2))
                    rT_ps = ps.tile([8, 128], F32, tag="ps", name=f"rTp{h}")
                    nc.tensor.transpose(rT_ps[:], ri2[:], ident_t[:])
                    rT_sb = small.tile([8, 128], F32, tag="rT", name=f"rT{h}")
                    nc.vector.tensor_copy(rT_sb[:], rT_ps[:])
                    rT1 = small.tile([1, L], F32, tag="rT1", name=f"rT1_{h}")
                    nc.gpsimd.dma_start(rT1[:], rT_sb[:])
                    repl_sb = replp.tile([64, L], F32, tag="repl",
                                         name=f"repl{h}")
                    rT1_rep = bass.AP(rT1.tensor, rT1.offset,
                                      [[1, 1], [0, 64], [1, L]])
                    nc.gpsimd.dma_start(repl_sb[:], rT1_rep)
                    pending_fin[0] = (pv_ps, repl_sb, h)

            emit_finalize()

            # ---- Phase C: out projection (partial over this core's heads)
            for lc in range(NT):
                op_ps = ps.tile([128, L], F32, tag="ps", name=f"op{lc}")
                for i in range(PAIRS):
                    st, sp_ = (i == 0), (i == PAIRS - 1)
                    asl = aot[i][:, lc * 128:(lc + 1) * 128]
                    nc.tensor.matmul(op_ps[:, 0:512], asl,
                                     wo_t[i][:, 0:512], start=st, stop=sp_)
                    nc.tensor.matmul(op_ps[:, 512:1024], asl,
                                     wo_t[i][:, 512:1024],
                                     start=st, stop=sp_)
                ot = xt.tile([128, L], F32, tag="xt", name=f"ot{lc}")
                nc.vector.tensor_copy(ot[:], op_ps[:])
                nc.sync.dma_start(outp[lc * 128:(lc + 1) * 128, :], ot[:])

    return nc


def kernel(q, k, v, Wq, bq, Wk, bk, Wv, bv, Wo, bo,
           lambda_q, lambda_k, lambda_v):
    global LAST_RESULT
    from concourse.bass_utils import run_bass_kernel_spmd

    q = np.asarray(q, F32NP)
    k = np.asarray(k, F32NP)
    v = np.asarray(v, F32NP)
    Wq = np.asarray(Wq, F32NP)
    Wk = np.asarray(Wk, F32NP)
    Wv = np.asarray(Wv, F32NP)
    Wo = np.asarray(Wo, F32NP)
    bq = np.asarray(bq, F32NP)
    bk = np.asarray(bk, F32NP)
    bv = np.asarray(bv, F32NP)
    bo = np.asarray(bo, F32NP)
    lambda_q = np.asarray(lambda_q, F32NP)
    lambda_k = np.asarray(lambda_k, F32NP)
    lambda_v = np.asarray(lambda_v, F32NP)

    lam_self = np.exp(np.sum(lambda_q * lambda_k))
    lam_cross = np.exp(np.sum(lambda_k * lambda_v))
    scale = F32NP(1.0) / F32NP(np.sqrt(HD))
    c1 = F32NP(scale * lam_cross)  # multiplies qh @ kh^T
    c2 = F32NP(scale * lam_self)   # multiplies kh @ vh^T

    nc = _build_program(1.0 / float(c2))

    ident = np.eye(128, dtype=F32NP)
    in_maps = []
    for c in range(NCORES):
        b = c // 2
        hs = (c % 2) * HPC
        ch = slice(hs * HD, (hs + HPC) * HD)  # natural channel slice
        # pair-swapped channel order for wv / bv
        sw_idx = np.concatenate([
            np.arange(hs * HD, (hs + HPC) * HD)
              .reshape(PAIRS, 2, HD)[:, ::-1, :].reshape(-1)
        ])
        in_maps.append({
            "xqT": np.ascontiguousarray(q[b].T),
            "xkT": np.ascontiguousarray(k[b].T),
            "xvT": np.ascontiguousarray(v[b].T),
            "wq": np.ascontiguousarray((c1 * Wq[ch, :]).T),
            "wk": np.ascontiguousarray(Wk[ch, :].T),
            "wv": np.ascontiguousarray((c2 * Wv[sw_idx, :]).T),
            "woT": np.ascontiguousarray(Wo[:, ch].T),
            "bq_d": np.ascontiguousarray(c1 * bq[ch]),
            "bk_d": np.ascontiguousarray(bk[ch]),
            "bv_d": np.ascontiguousarray(c2 * bv[sw_idx]),
            "ident_d": ident,
        })

    if TRACE:
        _ensure_ntff_hook()
    res = run_bass_kernel_spmd(nc, in_maps, core_ids=list(range(NCORES)),
                               trace=TRACE)
    LAST_RESULT = res

    attn = np.empty((B, H, L, L), F32NP)
    out = np.empty((B, L, D), F32NP)
    for c in range(NCORES):
        b = c // 2
        hs = (c % 2) * HPC
        attn[b, hs:hs + HPC] = res.results[c]["attn_w"]
    for b in range(B):
        out[b] = res.results[2 * b]["outp"] + res.results[2 * b + 1]["outp"]
    out += bv @ Wo.T + bo  # exact v-bias correction + output bias
    return out, attn


# revision 28
# speedup vs baseline: 1.1100x; 1.1100x over previous
"""AggAttn Trainium2 kernel: fused dual-score attention across 8 NeuronCores.

Sharding: core c handles batch b = c // 2 and heads (c % 2) * 8 .. + 8.
Host pre-transposes per-core inputs, folds the scalar gates into the
projection weights, launches one SPMD Bass program on cores 0-7, and
reassembles / reduces the partial outputs.

Math (per batch b, head h):
  qh = q @ Wq.T + bq   (per-head slices, [L, hd])
  S  = lam_cross/8 * qh @ kh.T + lam_self/8 * kh @ vh.T        [L, L]
  attn = softmax(S)  (returned as output 2)
  out  = concat_h(attn @ vh) @ Wo.T + bo                       [L, D]

Device-side formulation per head:
  A = [c1*qh^T ; kh^T]  (stacked [128, L]),  B = [kh^T ; c2*vh^T]
  S  chunk  = A[:, lc]^T @ B      (one K=128 matmul pair, both terms fused)
  S^T chunk = B[:, mc]^T @ A      (same product, transposed layout)
  exp(S) -> rowsum (ACT accum) -> attn = exp(S)/rowsum -> DRAM
  pv = vh_chunks^T-free PV: attn_out^T[hd, L] = sum_m (c2*vh)[m,:]^T exp(S^T)[m,:]
  attn_out^T normalized by 1/(c2*rowsum) broadcast, then row-parallel out-proj.
"""

import numpy as np

B, L, D, H, HD = 4, 1024, 1024, 16, 64
NCORES = 8
HPC = 8  # heads per core
PAIRS = 4  # head pairs per core
NT = 8  # 128-row tiles per 1024
F32NP = np.float32

# test.py knobs (the grading harness just calls kernel())
TRACE = False
LAST_RESULT = None  # BassKernelResults of the last run


def _ensure_ntff_hook():
    """Provide antenv.axon_hooks when the image lacks it, wiring the
    NTFF profile hook straight to libaxon_pjrt.so (slim copy of the
    trn_boot._ntff_profile_via_ctypes path)."""
    import contextlib
    import ctypes
    import sys
    import types

    try:
        from antenv.axon_hooks import get_axon_ntff_profile_hook  # noqa: F401
        return
    except ImportError:
        pass
    import antenv

    mod = types.ModuleType("antenv.axon_hooks")
    holder = [None]
    mod.set_axon_ntff_profile_hook = lambda h: holder.__setitem__(0, h)
    mod.get_axon_ntff_profile_hook = lambda: holder[0]
    sys.modules["antenv.axon_hooks"] = mod
    antenv.axon_hooks = mod

    so_path = "/opt/axon/libaxon_pjrt.so"
    try:
        lib = ctypes.CDLL(so_path)
    except OSError:
        return
    if not hasattr(lib, "axon_start_nrt_profile"):
        return
    lib.axon_start_nrt_profile.argtypes = [ctypes.POINTER(ctypes.c_int64),
                                           ctypes.c_size_t]
    lib.axon_start_nrt_profile.restype = ctypes.c_int64
    lib.axon_stop_nrt_profile.argtypes = [ctypes.c_char_p]
    lib.axon_stop_nrt_profile.restype = ctypes.c_int64

    @contextlib.contextmanager
    def _hook(output_dir, device_ids):
        import jax

        jax.devices()
        if device_ids:
            ids = (ctypes.c_int64 * len(device_ids))(*device_ids)
            rc = lib.axon_start_nrt_profile(ids, len(device_ids))
        else:
            rc = lib.axon_start_nrt_profile(None, 0)
        if rc != 0:
            raise RuntimeError(f"axon_start_nrt_profile rc={rc}")
        try:
            yield
        finally:
            n = lib.axon_stop_nrt_profile(str(output_dir).encode())
            print(f"ntff profile: {n} file(s) -> {output_dir}")

    mod.set_axon_ntff_profile_hook(_hook)


def _patch_tile_drain():
    """This image's walrus rejects instructions with >2 sync waits; Tile's
    kernel-tail drain collects one wait per live producer. Split them into
    a chain of single-wait nops on SyncE before the drain."""
    import concourse.tile as tile_mod
    from concourse.vector_clock import ScopedClock
    from concourse import mybir

    if getattr(tile_mod.TileContext, "_drain_split_patch", False):
        return

    def _drain_and_barrier(self, tick_clock, wait_clock):
        nc = self.nc
        w_nop = nc.sync.nop(nofuse=True, hint="drain_waits")
        wait_clock.add_sem_waits(
            w_nop.ins, ScopedClock({None: tick_clock.global_clock}))
        si = w_nop.ins.sync_info
        if si is not None and si.on_wait and len(si.on_wait) > 1:
            waits = list(si.on_wait)
            w_nop.ins.sync_info = mybir.SyncInfo(
                on_wait=waits[:1], on_update=list(si.on_update))
            for w in waits[1:]:
                n2 = nc.sync.nop(nofuse=True, hint="drain_waits")
                n2.ins.sync_info = mybir.SyncInfo(on_wait=[w], on_update=[])
        nc.sync.drain()
        nc.all_engine_barrier()
        assert self.sems is not None
        popped = nc._tile_sem_poison_stack.pop()
        assert popped is self._sem_poison
        nc.clear_and_free_semaphores(list(self.sems.allocated().values()))
        nc.all_engine_barrier()

    tile_mod.TileContext._drain_and_barrier = _drain_and_barrier
    tile_mod.TileContext._drain_split_patch = True

    # This walrus also caps EVERY instruction at one sync wait. Before
    # lowering, hoist excess waits onto same-engine single-wait nops
    # inserted immediately before the overloaded instruction.
    import bass_rust

    _orig_loi = tile_mod.TileContext._lower_ordered_insts
    _ctr = [0]

    def _split_waits_loi(self, ordered):
        for insts in ordered.values():
            out = []
            for inst in insts:
                si = getattr(inst, "sync_info", None)
                if (si is not None and si.on_wait and len(si.on_wait) > 1
                        and not hasattr(inst, "child_blocks")
                        and type(inst).__name__ not in
                        ("TileBranchInst", "BassTileLoopBlock")):
                    waits = list(si.on_wait)
                    for w in waits[:-1]:
                        _ctr[0] += 1
                        nop = bass_rust.InstNoOp(
                            name=f"I-wsplit{_ctr[0]}", ins=[], outs=[])
                        nop.engine = inst.engine
                        nop.bass_nofuse = True
                        nop.sync_info = mybir.SyncInfo(on_wait=[w],
                                                       on_update=[])
                        out.append(nop)
                    inst.sync_info = mybir.SyncInfo(
                        on_wait=[waits[-1]], on_update=list(si.on_update))
                out.append(inst)
            insts[:] = out
        return _orig_loi(self, ordered)

    tile_mod.TileContext._lower_ordered_insts = _split_waits_loi


def _build_program(inv_c2: float):
    import concourse.bass as bass
    import concourse.tile as tile
    from concourse import mybir
    from contextlib import ExitStack

    _patch_tile_drain()

    F32 = mybir.dt.float32
    F32R = mybir.dt.float32r
    AF = mybir.ActivationFunctionType
    OP = mybir.AluOpType

    nc = bass.Bass("TRN2", target_bir_lowering=False, debug=False,
                   num_devices=NCORES)

    xqT = nc.dram_tensor("xqT", [D, L], F32R, kind="ExternalInput").ap()
    xkT = nc.dram_tensor("xkT", [D, L], F32R, kind="ExternalInput").ap()
    xvT = nc.dram_tensor("xvT", [D, L], F32R, kind="ExternalInput").ap()
    wq = nc.dram_tensor("wq", [D, HPC * HD], F32R, kind="ExternalInput").ap()
    wk = nc.dram_tensor("wk", [D, HPC * HD], F32R, kind="ExternalInput").ap()
    wv = nc.dram_tensor("wv", [D, HPC * HD], F32R, kind="ExternalInput").ap()
    woT = nc.dram_tensor("woT", [HPC * HD, D], F32R, kind="ExternalInput").ap()
    bq_d = nc.dram_tensor("bq_d", [HPC * HD], F32, kind="ExternalInput").ap()
    bk_d = nc.dram_tensor("bk_d", [HPC * HD], F32, kind="ExternalInput").ap()
    bv_d = nc.dram_tensor("bv_d", [HPC * HD], F32, kind="ExternalInput").ap()
    ident_d = nc.dram_tensor("ident_d", [128, 128], F32,
                             kind="ExternalInput").ap()

    attn_w = nc.dram_tensor("attn_w", [HPC, L, L], F32,
                            kind="ExternalOutput").ap()
    outp = nc.dram_tensor("outp", [L, D], F32, kind="ExternalOutput").ap()

    with tile.TileContext(nc) as tc:
        with ExitStack() as ctx:
            xt = ctx.enter_context(tc.tile_pool(name="xt", bufs=15))
            wp = ctx.enter_context(tc.tile_pool(name="wp", bufs=8))
            stack = ctx.enter_context(tc.tile_pool(name="stack", bufs=1))
            vhp = ctx.enter_context(tc.tile_pool(name="vhp", bufs=1))
            wop = ctx.enter_context(tc.tile_pool(name="wop", bufs=1))
            aotp = ctx.enter_context(tc.tile_pool(name="aotp", bufs=1))
            small = ctx.enter_context(tc.tile_pool(name="small", bufs=2))
            replp = ctx.enter_context(tc.tile_pool(name="replp", bufs=2))
            const = ctx.enter_context(tc.tile_pool(name="const", bufs=1))
            ps = ctx.enter_context(
                tc.tile_pool(name="ps", bufs=3, space="PSUM"))
            ps_pv = ctx.enter_context(
                tc.tile_pool(name="ps_pv", bufs=1, space="PSUM"))

            # constants
            ident_t = const.tile([128, 128], F32, tag="ident", name="ident_t")
            nc.sync.dma_start(ident_t[:], ident_d[:])
            # PE warmup: ~17us of dummy matmuls during the initial input
            # loads, so HAM is at full clock when the first projection runs
            warm_ps = ps.tile([128, 128], F32, tag="ps", name="warm_ps")
            for _ in range(110):
                nc.tensor.matmul(warm_ps[:], ident_t[:], ident_t[:],
                                 start=True, stop=True)
            bq_t = const.tile([128, PAIRS], F32, tag="bq", name="bq_t")
            nc.sync.dma_start(bq_t[:], bq_d.rearrange("(a p) -> p a", p=128))
            bk_t = const.tile([128, PAIRS], F32, tag="bk", name="bk_t")
            nc.sync.dma_start(bk_t[:], bk_d.rearrange("(a p) -> p a", p=128))
            bv_t = const.tile([128, PAIRS], F32, tag="bv", name="bv_t")
            nc.sync.dma_start(bv_t[:], bv_d.rearrange("(a p) -> p a", p=128))

            # persistent tiles
            A = [stack.tile([128, L], F32R, tag=f"A{h}", name=f"A{h}")
                 for h in range(HPC)]
            Bs = [stack.tile([128, L], F32R, tag=f"B{h}", name=f"B{h}")
                  for h in range(HPC)]
            vh = [vhp.tile([128, HPC * HD], F32R, tag=f"vh{i}", name=f"vh{i}")
                  for i in range(NT)]
            aot = [aotp.tile([128, L], F32R, tag=f"aot{i}", name=f"aot{i}")
                   for i in range(PAIRS)]
            wo_t = [wop.tile([128, L], F32R, tag=f"wo{i}", name=f"wo{i}")
                    for i in range(PAIRS)]

            def load_xw(x_ap, w_ap, nm):
                # interleave x (SP queue) and w (ACT queue) chunk loads so
                # the first projection matmul can start after chunk 0 lands
                xs, ws = [], []
                for kc in range(NT):
                    t = xt.tile([128, L], F32R, tag="xt", name=f"x{nm}{kc}")
                    nc.sync.dma_start(t[:], x_ap[kc * 128:(kc + 1) * 128, :])
                    xs.append(t)
                    t2 = wp.tile([128, HPC * HD], F32R, tag="w",
                                 name=f"w{nm}{kc}")
                    nc.scalar.dma_start(t2[:],
                                        w_ap[kc * 128:(kc + 1) * 128, :])
                    ws.append(t2)
                return xs, ws

            def proj_pair(xts, wts, p, name):
                pp = ps.tile([128, L], F32, tag="ps", name=f"pp{name}{p}")
                for kc in range(NT):
                    st, sp_ = (kc == 0), (kc == NT - 1)
                    wsl = wts[kc][:, p * 128:(p + 1) * 128]
                    nc.tensor.matmul(pp[:, 0:512], wsl, xts[kc][:, 0:512],
                                     start=st, stop=sp_)
                    nc.tensor.matmul(pp[:, 512:1024], wsl,
                                     xts[kc][:, 512:1024],
                                     start=st, stop=sp_)
                return pp

            # ---- Phase A: projections into stacks ----
            xq_ts, wq_ts = load_xw(xqT, wq, "q")
            for p in range(PAIRS):
                pp = proj_pair(xq_ts, wq_ts, p, "q")
                h0, h1 = 2 * p, 2 * p + 1
                nc.vector.tensor_scalar_add(A[h0][0:64, :], pp[0:64, :],
                                            bq_t[0:64, p:p + 1])
                nc.vector.tensor_scalar_add(A[h1][64:128, :], pp[64:128, :],
                                            bq_t[64:128, p:p + 1])

            xk_ts, wk_ts = load_xw(xkT, wk, "k")
            for p in range(PAIRS):
                pp = proj_pair(xk_ts, wk_ts, p, "k")
                h0, h1 = 2 * p, 2 * p + 1
                nc.vector.tensor_scalar_add(Bs[h0][0:64, :], pp[0:64, :],
                                            bk_t[0:64, p:p + 1])
                nc.vector.tensor_scalar_add(Bs[h1][64:128, :], pp[64:128, :],
                                            bk_t[64:128, p:p + 1])
                # kh also needed in the A stacks (other partition half):
                nc.gpsimd.dma_start(A[h0][64:128, :], Bs[h0][0:64, :])
                nc.gpsimd.dma_start(A[h1][0:64, :], Bs[h1][64:128, :])

            # v: wv columns are pair-swapped on the host (head 2p+1 first)
            xv_ts, wv_ts = load_xw(xvT, wv, "v")
            for p in range(PAIRS):
                pp = proj_pair(xv_ts, wv_ts, p, "v1")
                h0, h1 = 2 * p, 2 * p + 1
                nc.vector.tensor_scalar_add(Bs[h1][0:64, :], pp[0:64, :],
                                            bv_t[0:64, p:p + 1])
                nc.vector.tensor_scalar_add(Bs[h0][64:128, :], pp[64:128, :],
                                            bv_t[64:128, p:p + 1])

            # v again, in [m, hd] layout for the PV matmul (no bias; the
            # bias term is exact on the host: softmax rows sum to 1).
            # Emitted inside head 0 (after its l-loop) so it overlaps the
            # l-loop's ACT work instead of delaying attention start.
            def emit_v2():
                for mc in range(NT):
                    pp = ps.tile([128, L], F32, tag="ps", name=f"ppv2{mc}")
                    for kc in range(NT):
                        nc.tensor.matmul(
                            pp[:, 0:512],
                            xv_ts[kc][:, mc * 128:(mc + 1) * 128],
                            wv_ts[kc][:],
                            start=(kc == 0), stop=(kc == NT - 1))
                    nc.vector.tensor_copy(vh[mc][:], pp[:, 0:512])

            emit_v2()

            for i in range(PAIRS):
                nc.scalar.dma_start(wo_t[i][:], woT[i * 128:(i + 1) * 128, :])

            # ---- Phase B: per-head attention ----
            # Interleaved chunk loop: S(lc) + S^T(mc) + PV(mc) together so
            # the PE stream stays dense (6 matmuls per chunk vs 2 exps on
            # ACT) and HAM stays at full clock.
            # finalize(h) = normalize attn_out^T and place into aot. Emitted
            # one head LATE (software pipeline) so the DVE's in-order stream
            # isn't blocked waiting for head h's repl DMA while head h+1's
            # l-loop DVE work is ready.
            pending_fin = [None]

            def emit_finalize():
                fin = pending_fin[0]
                if fin is None:
                    return
                pv_prev, repl_prev, hprev = fin
                pair_i, half = hprev // 2, hprev % 2
                if half == 0:
                    nc.vector.tensor_mul(aot[pair_i][0:64, :], pv_prev[:],
                                         repl_prev[:])
                else:
                    tmp = xt.tile([64, L], F32R, tag="xt",
                                  name=f"tmp{hprev}")
                    nc.vector.tensor_mul(tmp[:], pv_prev[:], repl_prev[:])
                    nc.gpsimd.dma_start(aot[pair_i][64:128, :], tmp[:])
                pending_fin[0] = None

            for h in range(HPC):
                A_, B_ = A[h], Bs[h]
                rs_t = small.tile([128, NT], F32, tag="rs", name=f"rs{h}")
                ri_t = small.tile([128, NT], F32, tag="ri", name=f"ri{h}")
                vcol = (h ^ 1) * HD
                with nc.named_scope(f"head{h}"):
                    # S side: attn output chunks
                    for i in range(NT):
                        sp_ = ps.tile([128, L], F32, tag="ps",
                                      name=f"s{h}_{i}")
                        asl = A_[:, i * 128:(i + 1) * 128]
                        nc.tensor.matmul(sp_[:, 0:512], asl, B_[:, 0:512],
                                         start=True, stop=True)
                        nc.tensor.matmul(sp_[:, 512:1024], asl,
                                         B_[:, 512:1024],
                                         start=True, stop=True)
                        ex = xt.tile([128, L], F32, tag="xt",
                                       name=f"ex{h}_{i}")
                        nc.scalar.activation(ex[:], sp_[:], AF.Exp,
                                             accum_out=rs_t[:, i:i + 1])
                        nc.vector.reciprocal(ri_t[:, i:i + 1],
                                             rs_t[:, i:i + 1])
                        at = xt.tile([128, L], F32, tag="xt",
                                       name=f"at{h}_{i}")
                        nc.vector.tensor_scalar_mul(at[:], ex[:],
                                                    ri_t[:, i:i + 1])
                        nc.sync.dma_start(
                            attn_w[h, i * 128:(i + 1) * 128, :], at[:])

                    # previous head's finalize: its repl DMA completed while
                    # this head's l-loop ran
                    emit_finalize()


                    # S^T chunks feeding the PV accumulation
                    pv_ps = ps_pv.tile([64, L], F32, tag="pv", name=f"pv{h}")
                    for i in range(NT):
                        stp = ps.tile([128, L], F32, tag="ps",
                                      name=f"st{h}_{i}")
                        bsl = B_[:, i * 128:(i + 1) * 128]
                        nc.tensor.matmul(stp[:, 0:512], bsl, A_[:, 0:512],
                                         start=True, stop=True)
                        nc.tensor.matmul(stp[:, 512:1024], bsl,
                                         A_[:, 512:1024],
                                         start=True, stop=True)
                        ext = xt.tile([128, L], F32R, tag="xt",
                                        name=f"ext{h}_{i}")
                        nc.scalar.activation(ext[:], stp[:], AF.Exp)
                        vsl = vh[i][:, vcol:vcol + HD]
                        nc.tensor.matmul(pv_ps[:, 0:512], vsl, ext[:, 0:512],
                                         start=(i == 0), stop=(i == NT - 1))
                        nc.tensor.matmul(pv_ps[:, 512:1024], vsl,
                                         ext[:, 512:1024],
                                         start=(i == 0), stop=(i == NT - 1))

                    # 1/(c2*rowsum) -> [64, L] replica; small DMAs ride the
                    # idle SWDGE ring so they never queue behind attn writes
                    ri2 = small.tile([128, NT], F32, tag="ri2",
                                     name=f"ri2_{h}")
                    nc.vector.tensor_scalar_mul(ri2[:], ri_t[:],
                                                float(inv_c2))
                    rT_ps = ps.tile([8, 128], F32, tag="ps", name=f"rTp{h}")
                    nc.tensor.transpose(rT_ps[:], ri2[:], ident_t[:])
                    rT_sb = small.tile([8, 128], F32, tag="rT", name=f"rT{h}")
                    nc.vector.tensor_copy(rT_sb[:], rT_ps[:])
                    rT1 = small.tile([1, L], F32, tag="rT1", name=f"rT1_{h}")
                    nc.gpsimd.dma_start(rT1[:], rT_sb[:])
                    repl_sb = replp.tile([64, L], F32, tag="repl",
                                         name=f"repl{h}")
                    rT1_rep = bass.AP(rT1.tensor, rT1.offset,
                                      [[1, 1], [0, 64], [1, L]])
                    nc.gpsimd.dma_start(repl_sb[:], rT1_rep)
                    pending_fin[0] = (pv_ps, repl_sb, h)

            emit_finalize()

            # ---- Phase C: out projection (partial over this core's heads)
            for lc in range(NT):
                op_ps = ps.tile([128, L], F32, tag="ps", name=f"op{lc}")
                for i in range(PAIRS):
                    st, sp_ = (i == 0), (i == PAIRS - 1)
                    asl = aot[i][:, lc * 128:(lc + 1) * 128]
                    nc.tensor.matmul(op_ps[:, 0:512], asl,
                                     wo_t[i][:, 0:512], start=st, stop=sp_)
                    nc.tensor.matmul(op_ps[:, 512:1024], asl,
                                     wo_t[i][:, 512:1024],
                                     start=st, stop=sp_)
                ot = xt.tile([128, L], F32, tag="xt", name=f"ot{lc}")
                nc.vector.tensor_copy(ot[:], op_ps[:])
                nc.sync.dma_start(outp[lc * 128:(lc + 1) * 128, :], ot[:])

    return nc


def kernel(q, k, v, Wq, bq, Wk, bk, Wv, bv, Wo, bo,
           lambda_q, lambda_k, lambda_v):
    global LAST_RESULT
    from concourse.bass_utils import run_bass_kernel_spmd

    q = np.asarray(q, F32NP)
    k = np.asarray(k, F32NP)
    v = np.asarray(v, F32NP)
    Wq = np.asarray(Wq, F32NP)
    Wk = np.asarray(Wk, F32NP)
    Wv = np.asarray(Wv, F32NP)
    Wo = np.asarray(Wo, F32NP)
    bq = np.asarray(bq, F32NP)
    bk = np.asarray(bk, F32NP)
    bv = np.asarray(bv, F32NP)
    bo = np.asarray(bo, F32NP)
    lambda_q = np.asarray(lambda_q, F32NP)
    lambda_k = np.asarray(lambda_k, F32NP)
    lambda_v = np.asarray(lambda_v, F32NP)

    lam_self = np.exp(np.sum(lambda_q * lambda_k))
    lam_cross = np.exp(np.sum(lambda_k * lambda_v))
    scale = F32NP(1.0) / F32NP(np.sqrt(HD))
    c1 = F32NP(scale * lam_cross)  # multiplies qh @ kh^T
    c2 = F32NP(scale * lam_self)   # multiplies kh @ vh^T

    nc = _build_program(1.0 / float(c2))

    ident = np.eye(128, dtype=F32NP)
    in_maps = []
    for c in range(NCORES):
        b = c // 2
        hs = (c % 2) * HPC
        ch = slice(hs * HD, (hs + HPC) * HD)  # natural channel slice
        # pair-swapped channel order for wv / bv
        sw_idx = np.concatenate([
            np.arange(hs * HD, (hs + HPC) * HD)
              .reshape(PAIRS, 2, HD)[:, ::-1, :].reshape(-1)
        ])
        in_maps.append({
            "xqT": np.ascontiguousarray(q[b].T),
            "xkT": np.ascontiguousarray(k[b].T),
            "xvT": np.ascontiguousarray(v[b].T),
            "wq": np.ascontiguousarray((c1 * Wq[ch, :]).T),
            "wk": np.ascontiguousarray(Wk[ch, :].T),
            "wv": np.ascontiguousarray((c2 * Wv[sw_idx, :]).T),
            "woT": np.ascontiguousarray(Wo[:, ch].T),
            "bq_d": np.ascontiguousarray(c1 * bq[ch]),
            "bk_d": np.ascontiguousarray(bk[ch]),
            "bv_d": np.ascontiguousarray(c2 * bv[sw_idx]),
            "ident_d": ident,
        })

    if TRACE:
        _ensure_ntff_hook()
    res = run_bass_kernel_spmd(nc, in_maps, core_ids=list(range(NCORES)),
                               trace=TRACE)
    LAST_RESULT = res

    attn = np.empty((B, H, L, L), F32NP)
    out = np.empty((B, L, D), F32NP)
    for c in range(NCORES):
        b = c // 2
        hs = (c % 2) * HPC
        attn[b, hs:hs + HPC] = res.results[c]["attn_w"]
    for b in range(B):
        out[b] = res.results[2 * b]["outp"] + res.results[2 * b + 1]["outp"]
    out += bv @ Wo.T + bo  # exact v-bias correction + output bias
    return out, attn


# revision 29
# speedup vs baseline: 1.1611x; 1.0460x over previous
"""AggAttn Trainium2 kernel: fused dual-score attention across 8 NeuronCores.

Sharding: core c handles batch b = c // 2 and heads (c % 2) * 8 .. + 8.
Host pre-transposes per-core inputs, folds the scalar gates into the
projection weights, launches one SPMD Bass program on cores 0-7, and
reassembles / reduces the partial outputs.

Math (per batch b, head h):
  qh = q @ Wq.T + bq   (per-head slices, [L, hd])
  S  = lam_cross/8 * qh @ kh.T + lam_self/8 * kh @ vh.T        [L, L]
  attn = softmax(S)  (returned as output 2)
  out  = concat_h(attn @ vh) @ Wo.T + bo                       [L, D]

Device-side formulation per head:
  A = [c1*qh^T ; kh^T]  (stacked [128, L]),  B = [kh^T ; c2*vh^T]
  S  chunk  = A[:, lc]^T @ B      (one K=128 matmul pair, both terms fused)
  S^T chunk = B[:, mc]^T @ A      (same product, transposed layout)
  exp(S) -> rowsum (ACT accum) -> attn = exp(S)/rowsum -> DRAM
  pv = vh_chunks^T-free PV: attn_out^T[hd, L] = sum_m (c2*vh)[m,:]^T exp(S^T)[m,:]
  attn_out^T normalized by 1/(c2*rowsum) broadcast, then row-parallel out-proj.
"""

import numpy as np

B, L, D, H, HD = 4, 1024, 1024, 16, 64
NCORES = 8
HPC = 8  # heads per core
PAIRS = 4  # head pairs per core
NT = 8  # 128-row tiles per 1024
F32NP = np.float32

# test.py knobs (the grading harness just calls kernel())
TRACE = False
LAST_RESULT = None  # BassKernelResults of the last run


def _ensure_ntff_hook():
    """Provide antenv.axon_hooks when the image lacks it, wiring the
    NTFF profile hook straight to libaxon_pjrt.so (slim copy of the
    trn_boot._ntff_profile_via_ctypes path)."""
    import contextlib
    import ctypes
    import sys
    import types

    try:
        from antenv.axon_hooks import get_axon_ntff_profile_hook  # noqa: F401
        return
    except ImportError:
        pass
    import antenv

    mod = types.ModuleType("antenv.axon_hooks")
    holder = [None]
    mod.set_axon_ntff_profile_hook = lambda h: holder.__setitem__(0, h)
    mod.get_axon_ntff_profile_hook = lambda: holder[0]
    sys.modules["antenv.axon_hooks"] = mod
    antenv.axon_hooks = mod

    so_path = "/opt/axon/libaxon_pjrt.so"
    try:
        lib = ctypes.CDLL(so_path)
    except OSError:
        return
    if not hasattr(lib, "axon_start_nrt_profile"):
        return
    lib.axon_start_nrt_profile.argtypes = [ctypes.POINTER(ctypes.c_int64),
                                           ctypes.c_size_t]
    lib.axon_start_nrt_profile.restype = ctypes.c_int64
    lib.axon_stop_nrt_profile.argtypes = [ctypes.c_char_p]
    lib.axon_stop_nrt_profile.restype = ctypes.c_int64

    @contextlib.contextmanager
    def _hook(output_dir, device_ids):
        import jax

        jax.devices()
        if device_ids:
            ids = (ctypes.c_int64 * len(device_ids))(*device_ids)
            rc = lib.axon_start_nrt_profile(ids, len(device_ids))
        else:
            rc = lib.axon_start_nrt_profile(None, 0)
        if rc != 0:
            raise RuntimeError(f"axon_start_nrt_profile rc={rc}")
        try:
            yield
        finally:
            n = lib.axon_stop_nrt_profile(str(output_dir).encode())
            print(f"ntff profile: {n} file(s) -> {output_dir}")

    mod.set_axon_ntff_profile_hook(_hook)


def _patch_tile_drain():
    """This image's walrus rejects instructions with >2 sync waits; Tile's
    kernel-tail drain collects one wait per live producer. Split them into
    a chain of single-wait nops on SyncE before the drain."""
    import concourse.tile as tile_mod
    from concourse.vector_clock import ScopedClock
    from concourse import mybir

    if getattr(tile_mod.TileContext, "_drain_split_patch", False):
        return

    def _drain_and_barrier(self, tick_clock, wait_clock):
        nc = self.nc
        w_nop = nc.sync.nop(nofuse=True, hint="drain_waits")
        wait_clock.add_sem_waits(
            w_nop.ins, ScopedClock({None: tick_clock.global_clock}))
        si = w_nop.ins.sync_info
        if si is not None and si.on_wait and len(si.on_wait) > 1:
            waits = list(si.on_wait)
            w_nop.ins.sync_info = mybir.SyncInfo(
                on_wait=waits[:1], on_update=list(si.on_update))
            for w in waits[1:]:
                n2 = nc.sync.nop(nofuse=True, hint="drain_waits")
                n2.ins.sync_info = mybir.SyncInfo(on_wait=[w], on_update=[])
        nc.sync.drain()
        nc.all_engine_barrier()
        assert self.sems is not None
        popped = nc._tile_sem_poison_stack.pop()
        assert popped is self._sem_poison
        nc.clear_and_free_semaphores(list(self.sems.allocated().values()))
        nc.all_engine_barrier()

    tile_mod.TileContext._drain_and_barrier = _drain_and_barrier
    tile_mod.TileContext._drain_split_patch = True

    # This walrus also caps EVERY instruction at one sync wait. Before
    # lowering, hoist excess waits onto same-engine single-wait nops
    # inserted immediately before the overloaded instruction.
    import bass_rust

    _orig_loi = tile_mod.TileContext._lower_ordered_insts
    _ctr = [0]

    def _split_waits_loi(self, ordered):
        for insts in ordered.values():
            out = []
            for inst in insts:
                si = getattr(inst, "sync_info", None)
                if (si is not None and si.on_wait and len(si.on_wait) > 1
                        and not hasattr(inst, "child_blocks")
                        and type(inst).__name__ not in
                        ("TileBranchInst", "BassTileLoopBlock")):
                    waits = list(si.on_wait)
                    for w in waits[:-1]:
                        _ctr[0] += 1
                        nop = bass_rust.InstNoOp(
                            name=f"I-wsplit{_ctr[0]}", ins=[], outs=[])
                        nop.engine = inst.engine
                        nop.bass_nofuse = True
                        nop.sync_info = mybir.SyncInfo(on_wait=[w],
                                                       on_update=[])
                        out.append(nop)
                    inst.sync_info = mybir.SyncInfo(
                        on_wait=[waits[-1]], on_update=list(si.on_update))
                out.append(inst)
            insts[:] = out
        return _orig_loi(self, ordered)

    tile_mod.TileContext._lower_ordered_insts = _split_waits_loi


def _build_program(inv_c2: float):
    import concourse.bass as bass
    import concourse.tile as tile
    from concourse import mybir
    from contextlib import ExitStack

    _patch_tile_drain()

    F32 = mybir.dt.float32
    F32R = mybir.dt.float32r
    AF = mybir.ActivationFunctionType
    OP = mybir.AluOpType

    nc = bass.Bass("TRN2", target_bir_lowering=False, debug=False,
                   num_devices=NCORES)

    BF16 = mybir.dt.bfloat16
    xqT = nc.dram_tensor("xqT", [D, L], BF16, kind="ExternalInput").ap()
    xkT = nc.dram_tensor("xkT", [D, L], BF16, kind="ExternalInput").ap()
    xvT = nc.dram_tensor("xvT", [D, L], BF16, kind="ExternalInput").ap()
    wq = nc.dram_tensor("wq", [D, HPC * HD], BF16, kind="ExternalInput").ap()
    wk = nc.dram_tensor("wk", [D, HPC * HD], BF16, kind="ExternalInput").ap()
    wv = nc.dram_tensor("wv", [D, HPC * HD], BF16, kind="ExternalInput").ap()
    woT = nc.dram_tensor("woT", [HPC * HD, D], F32R, kind="ExternalInput").ap()
    bq_d = nc.dram_tensor("bq_d", [HPC * HD], F32, kind="ExternalInput").ap()
    bk_d = nc.dram_tensor("bk_d", [HPC * HD], F32, kind="ExternalInput").ap()
    bv_d = nc.dram_tensor("bv_d", [HPC * HD], F32, kind="ExternalInput").ap()
    ident_d = nc.dram_tensor("ident_d", [128, 128], F32,
                             kind="ExternalInput").ap()

    attn_w = nc.dram_tensor("attn_w", [HPC, L, L], F32,
                            kind="ExternalOutput").ap()
    outp = nc.dram_tensor("outp", [L, D], F32, kind="ExternalOutput").ap()

    with tile.TileContext(nc) as tc:
        with ExitStack() as ctx:
            xt = ctx.enter_context(tc.tile_pool(name="xt", bufs=15))
            wp = ctx.enter_context(tc.tile_pool(name="wp", bufs=8))
            stack = ctx.enter_context(tc.tile_pool(name="stack", bufs=1))
            vhp = ctx.enter_context(tc.tile_pool(name="vhp", bufs=1))
            wop = ctx.enter_context(tc.tile_pool(name="wop", bufs=1))
            aotp = ctx.enter_context(tc.tile_pool(name="aotp", bufs=1))
            small = ctx.enter_context(tc.tile_pool(name="small", bufs=2))
            replp = ctx.enter_context(tc.tile_pool(name="replp", bufs=2))
            const = ctx.enter_context(tc.tile_pool(name="const", bufs=1))
            ps = ctx.enter_context(
                tc.tile_pool(name="ps", bufs=3, space="PSUM"))
            ps_pv = ctx.enter_context(
                tc.tile_pool(name="ps_pv", bufs=1, space="PSUM"))

            # constants
            ident_t = const.tile([128, 128], F32, tag="ident", name="ident_t")
            nc.sync.dma_start(ident_t[:], ident_d[:])
            # PE warmup: ~17us of dummy matmuls during the initial input
            # loads, so HAM is at full clock when the first projection runs
            warm_ps = ps.tile([128, 128], F32, tag="ps", name="warm_ps")
            for _ in range(40):
                nc.tensor.matmul(warm_ps[:], ident_t[:], ident_t[:],
                                 start=True, stop=True)
            bq_t = const.tile([128, PAIRS], F32, tag="bq", name="bq_t")
            nc.sync.dma_start(bq_t[:], bq_d.rearrange("(a p) -> p a", p=128))
            bk_t = const.tile([128, PAIRS], F32, tag="bk", name="bk_t")
            nc.sync.dma_start(bk_t[:], bk_d.rearrange("(a p) -> p a", p=128))
            bv_t = const.tile([128, PAIRS], F32, tag="bv", name="bv_t")
            nc.sync.dma_start(bv_t[:], bv_d.rearrange("(a p) -> p a", p=128))

            # persistent tiles
            A = [stack.tile([128, L], F32R, tag=f"A{h}", name=f"A{h}")
                 for h in range(HPC)]
            Bs = [stack.tile([128, L], F32R, tag=f"B{h}", name=f"B{h}")
                  for h in range(HPC)]
            vh = [vhp.tile([128, HPC * HD], F32R, tag=f"vh{i}", name=f"vh{i}")
                  for i in range(NT)]
            aot = [aotp.tile([128, L], F32R, tag=f"aot{i}", name=f"aot{i}")
                   for i in range(PAIRS)]
            wo_t = [wop.tile([128, L], F32R, tag=f"wo{i}", name=f"wo{i}")
                    for i in range(PAIRS)]

            def load_xw(x_ap, w_ap, nm):
                # interleave x (SP queue) and w (ACT queue) chunk loads so
                # the first projection matmul can start after chunk 0 lands
                xs, ws = [], []
                for kc in range(NT):
                    t = xt.tile([128, L], BF16, tag="xt", name=f"x{nm}{kc}")
                    nc.sync.dma_start(t[:], x_ap[kc * 128:(kc + 1) * 128, :])
                    xs.append(t)
                    t2 = wp.tile([128, HPC * HD], BF16, tag="w",
                                 name=f"w{nm}{kc}")
                    nc.scalar.dma_start(t2[:],
                                        w_ap[kc * 128:(kc + 1) * 128, :])
                    ws.append(t2)
                return xs, ws

            def proj_pair(xts, wts, p, name):
                pp = ps.tile([128, L], F32, tag="ps", name=f"pp{name}{p}")
                for kc in range(NT):
                    st, sp_ = (kc == 0), (kc == NT - 1)
                    wsl = wts[kc][:, p * 128:(p + 1) * 128]
                    nc.tensor.matmul(pp[:, 0:512], wsl, xts[kc][:, 0:512],
                                     start=st, stop=sp_)
                    nc.tensor.matmul(pp[:, 512:1024], wsl,
                                     xts[kc][:, 512:1024],
                                     start=st, stop=sp_)
                return pp

            # ---- Phase A: projections into stacks ----
            xq_ts, wq_ts = load_xw(xqT, wq, "q")
            for p in range(PAIRS):
                pp = proj_pair(xq_ts, wq_ts, p, "q")
                h0, h1 = 2 * p, 2 * p + 1
                nc.vector.tensor_scalar_add(A[h0][0:64, :], pp[0:64, :],
                                            bq_t[0:64, p:p + 1])
                nc.vector.tensor_scalar_add(A[h1][64:128, :], pp[64:128, :],
                                            bq_t[64:128, p:p + 1])

            xk_ts, wk_ts = load_xw(xkT, wk, "k")
            for p in range(PAIRS):
                pp = proj_pair(xk_ts, wk_ts, p, "k")
                h0, h1 = 2 * p, 2 * p + 1
                nc.vector.tensor_scalar_add(Bs[h0][0:64, :], pp[0:64, :],
                                            bk_t[0:64, p:p + 1])
                nc.vector.tensor_scalar_add(Bs[h1][64:128, :], pp[64:128, :],
                                            bk_t[64:128, p:p + 1])
                # kh also needed in the A stacks (other partition half):
                nc.gpsimd.dma_start(A[h0][64:128, :], Bs[h0][0:64, :])
                nc.gpsimd.dma_start(A[h1][0:64, :], Bs[h1][64:128, :])

            # v: wv columns are pair-swapped on the host (head 2p+1 first)
            xv_ts, wv_ts = load_xw(xvT, wv, "v")
            for p in range(PAIRS):
                pp = proj_pair(xv_ts, wv_ts, p, "v1")
                h0, h1 = 2 * p, 2 * p + 1
                nc.vector.tensor_scalar_add(Bs[h1][0:64, :], pp[0:64, :],
                                            bv_t[0:64, p:p + 1])
                nc.vector.tensor_scalar_add(Bs[h0][64:128, :], pp[64:128, :],
                                            bv_t[64:128, p:p + 1])

            # v again, in [m, hd] layout for the PV matmul (no bias; the
            # bias term is exact on the host: softmax rows sum to 1).
            # Emitted inside head 0 (after its l-loop) so it overlaps the
            # l-loop's ACT work instead of delaying attention start.
            def emit_v2():
                for mc in range(NT):
                    pp = ps.tile([128, L], F32, tag="ps", name=f"ppv2{mc}")
                    for kc in range(NT):
                        nc.tensor.matmul(
                            pp[:, 0:512],
                            xv_ts[kc][:, mc * 128:(mc + 1) * 128],
                            wv_ts[kc][:],
                            start=(kc == 0), stop=(kc == NT - 1))
                    nc.vector.tensor_copy(vh[mc][:], pp[:, 0:512])

            emit_v2()

            for i in range(PAIRS):
                nc.scalar.dma_start(wo_t[i][:], woT[i * 128:(i + 1) * 128, :])

            # ---- Phase B: per-head attention ----
            # Interleaved chunk loop: S(lc) + S^T(mc) + PV(mc) together so
            # the PE stream stays dense (6 matmuls per chunk vs 2 exps on
            # ACT) and HAM stays at full clock.
            # finalize(h) = normalize attn_out^T and place into aot. Emitted
            # one head LATE (software pipeline) so the DVE's in-order stream
            # isn't blocked waiting for head h's repl DMA while head h+1's
            # l-loop DVE work is ready.
            pending_fin = [None]

            def emit_finalize():
                fin = pending_fin[0]
                if fin is None:
                    return
                pv_prev, repl_prev, hprev = fin
                pair_i, half = hprev // 2, hprev % 2
                if half == 0:
                    nc.vector.tensor_mul(aot[pair_i][0:64, :], pv_prev[:],
                                         repl_prev[:])
                else:
                    tmp = xt.tile([64, L], F32R, tag="xt",
                                  name=f"tmp{hprev}")
                    nc.vector.tensor_mul(tmp[:], pv_prev[:], repl_prev[:])
                    nc.gpsimd.dma_start(aot[pair_i][64:128, :], tmp[:])
                pending_fin[0] = None

            for h in range(HPC):
                A_, B_ = A[h], Bs[h]
                rs_t = small.tile([128, NT], F32, tag="rs", name=f"rs{h}")
                ri_t = small.tile([128, NT], F32, tag="ri", name=f"ri{h}")
                vcol = (h ^ 1) * HD
                with nc.named_scope(f"head{h}"):
                    # S side: attn output chunks
                    for i in range(NT):
                        sp_ = ps.tile([128, L], F32, tag="ps",
                                      name=f"s{h}_{i}")
                        asl = A_[:, i * 128:(i + 1) * 128]
                        nc.tensor.matmul(sp_[:, 0:512], asl, B_[:, 0:512],
                                         start=True, stop=True)
                        nc.tensor.matmul(sp_[:, 512:1024], asl,
                                         B_[:, 512:1024],
                                         start=True, stop=True)
                        ex = xt.tile([128, L], F32, tag="xt",
                                       name=f"ex{h}_{i}")
                        nc.scalar.activation(ex[:], sp_[:], AF.Exp,
                                             accum_out=rs_t[:, i:i + 1])
                        nc.vector.reciprocal(ri_t[:, i:i + 1],
                                             rs_t[:, i:i + 1])
                        at = xt.tile([128, L], F32, tag="xt",
                                       name=f"at{h}_{i}")
                        nc.vector.tensor_scalar_mul(at[:], ex[:],
                                                    ri_t[:, i:i + 1])
                        nc.sync.dma_start(
                            attn_w[h, i * 128:(i + 1) * 128, :], at[:])

                    # previous head's finalize: its repl DMA completed while
                    # this head's l-loop ran
                    emit_finalize()


                    # S^T chunks feeding the PV accumulation
                    pv_ps = ps_pv.tile([64, L], F32, tag="pv", name=f"pv{h}")
                    for i in range(NT):
                        stp = ps.tile([128, L], F32, tag="ps",
                                      name=f"st{h}_{i}")
                        bsl = B_[:, i * 128:(i + 1) * 128]
                        nc.tensor.matmul(stp[:, 0:512], bsl, A_[:, 0:512],
                                         start=True, stop=True)
                        nc.tensor.matmul(stp[:, 512:1024], bsl,
                                         A_[:, 512:1024],
                                         start=True, stop=True)
                        ext = xt.tile([128, L], F32R, tag="xt",
                                        name=f"ext{h}_{i}")
                        nc.scalar.activation(ext[:], stp[:], AF.Exp)
                        vsl = vh[i][:, vcol:vcol + HD]
                        nc.tensor.matmul(pv_ps[:, 0:512], vsl, ext[:, 0:512],
                                         start=(i == 0), stop=(i == NT - 1))
                        nc.tensor.matmul(pv_ps[:, 512:1024], vsl,
                                         ext[:, 512:1024],
                                         start=(i == 0), stop=(i == NT - 1))

                    # 1/(c2*rowsum) -> [64, L] replica; small DMAs ride the
                    # idle SWDGE ring so they never queue behind attn writes
                    ri2 = small.tile([128, NT], F32, tag="ri2",
                                     name=f"ri2_{h}")
                    nc.vector.tensor_scalar_mul(ri2[:], ri_t[:],
                                                float(inv_c2))
                    rT_ps = ps.tile([8, 128], F32, tag="ps", name=f"rTp{h}")
                    nc.tensor.transpose(rT_ps[:], ri2[:], ident_t[:])
                    rT_sb = small.tile([8, 128], F32, tag="rT", name=f"rT{h}")
                    nc.vector.tensor_copy(rT_sb[:], rT_ps[:])
                    rT1 = small.tile([1, L], F32, tag="rT1", name=f"rT1_{h}")
                    nc.gpsimd.dma_start(rT1[:], rT_sb[:])
                    repl_sb = replp.tile([64, L], F32, tag="repl",
                                         name=f"repl{h}")
                    rT1_rep = bass.AP(rT1.tensor, rT1.offset,
                                      [[1, 1], [0, 64], [1, L]])
                    nc.gpsimd.dma_start(repl_sb[:], rT1_rep)
                    pending_fin[0] = (pv_ps, repl_sb, h)

            emit_finalize()

            # ---- Phase C: out projection (partial over this core's heads)
            for lc in range(NT):
                op_ps = ps.tile([128, L], F32, tag="ps", name=f"op{lc}")
                for i in range(PAIRS):
                    st, sp_ = (i == 0), (i == PAIRS - 1)
                    asl = aot[i][:, lc * 128:(lc + 1) * 128]
                    nc.tensor.matmul(op_ps[:, 0:512], asl,
                                     wo_t[i][:, 0:512], start=st, stop=sp_)
                    nc.tensor.matmul(op_ps[:, 512:1024], asl,
                                     wo_t[i][:, 512:1024],
                                     start=st, stop=sp_)
                ot = xt.tile([128, L], F32, tag="xt", name=f"ot{lc}")
                nc.vector.tensor_copy(ot[:], op_ps[:])
                nc.sync.dma_start(outp[lc * 128:(lc + 1) * 128, :], ot[:])

    return nc


def kernel(q, k, v, Wq, bq, Wk, bk, Wv, bv, Wo, bo,
           lambda_q, lambda_k, lambda_v):
    global LAST_RESULT
    from concourse.bass_utils import run_bass_kernel_spmd

    q = np.asarray(q, F32NP)
    k = np.asarray(k, F32NP)
    v = np.asarray(v, F32NP)
    Wq = np.asarray(Wq, F32NP)
    Wk = np.asarray(Wk, F32NP)
    Wv = np.asarray(Wv, F32NP)
    Wo = np.asarray(Wo, F32NP)
    bq = np.asarray(bq, F32NP)
    bk = np.asarray(bk, F32NP)
    bv = np.asarray(bv, F32NP)
    bo = np.asarray(bo, F32NP)
    lambda_q = np.asarray(lambda_q, F32NP)
    lambda_k = np.asarray(lambda_k, F32NP)
    lambda_v = np.asarray(lambda_v, F32NP)

    lam_self = np.exp(np.sum(lambda_q * lambda_k))
    lam_cross = np.exp(np.sum(lambda_k * lambda_v))
    scale = F32NP(1.0) / F32NP(np.sqrt(HD))
    c1 = F32NP(scale * lam_cross)  # multiplies qh @ kh^T
    c2 = F32NP(scale * lam_self)   # multiplies kh @ vh^T

    nc = _build_program(1.0 / float(c2))

    ident = np.eye(128, dtype=F32NP)
    in_maps = []
    for c in range(NCORES):
        b = c // 2
        hs = (c % 2) * HPC
        ch = slice(hs * HD, (hs + HPC) * HD)  # natural channel slice
        # pair-swapped channel order for wv / bv
        sw_idx = np.concatenate([
            np.arange(hs * HD, (hs + HPC) * HD)
              .reshape(PAIRS, 2, HD)[:, ::-1, :].reshape(-1)
        ])
        import ml_dtypes
        bf16 = ml_dtypes.bfloat16
        in_maps.append({
            "xqT": np.ascontiguousarray(q[b].T.astype(bf16)),
            "xkT": np.ascontiguousarray(k[b].T.astype(bf16)),
            "xvT": np.ascontiguousarray(v[b].T.astype(bf16)),
            "wq": np.ascontiguousarray((c1 * Wq[ch, :]).T.astype(bf16)),
            "wk": np.ascontiguousarray(Wk[ch, :].T.astype(bf16)),
            "wv": np.ascontiguousarray((c2 * Wv[sw_idx, :]).T.astype(bf16)),
            "woT": np.ascontiguousarray(Wo[:, ch].T),
            "bq_d": np.ascontiguousarray(c1 * bq[ch]),
            "bk_d": np.ascontiguousarray(bk[ch]),
            "bv_d": np.ascontiguousarray(c2 * bv[sw_idx]),
            "ident_d": ident,
        })

    if TRACE:
        _ensure_ntff_hook()
    res = run_bass_kernel_spmd(nc, in_maps, core_ids=list(range(NCORES)),
                               trace=TRACE)
    LAST_RESULT = res

    attn = np.empty((B, H, L, L), F32NP)
    out = np.empty((B, L, D), F32NP)
    for c in range(NCORES):
        b = c // 2
        hs = (c % 2) * HPC
        attn[b, hs:hs + HPC] = res.results[c]["attn_w"]
    for b in range(B):
        out[b] = res.results[2 * b]["outp"] + res.results[2 * b + 1]["outp"]
    out += bv @ Wo.T + bo  # exact v-bias correction + output bias
    return out, attn
